# revision 12
# baseline (speedup 1.0000x reference)
"""Trainium2 Bass kernel for a gated linear-RNN block (RNNBlock).

Self-contained: takes FULL inputs (as produced by the problem's setup_inputs),
distributes across 8 NeuronCores internally, returns the FULL output.

Sharding:
  phase 0: token-parallel rmsnorm (each core 512 of 4096 flat tokens),
           AllGather xn^T (bf16)
  phase 1: head-parallel (core c <-> head c) projections + chunked gated
           linear RNN (chunk=128, log-space decay)
  phase 2: AllToAll og^T, then token-parallel Wo + residual + rmsnorm2 +
           SwiGLU MLP
Activations on device are feature-major ([feature, token]).  Host does layout
transforms (transpose/cast/concat) and folds the norm gain vectors and the
1/sqrt(DK) constant into weight matrices.
"""
import sys

for _p in ("/opt/trn_rl_repo", "/root/.axon_site/_ro/trn_rl_repo"):
    if _p not in sys.path:
        sys.path.append(_p)

import numpy as np
import ml_dtypes

import concourse.bass as bass
import concourse.mybir as mybir
import concourse.tile as tile
from concourse import bacc
from concourse import bass_utils

dt = mybir.dt
AF = mybir.ActivationFunctionType
ALU = mybir.AluOpType
BF = ml_dtypes.bfloat16

B, S, D, H, DK, DV, FF = 2, 2048, 1024, 8, 128, 128, 2816
T = B * S               # 4096 flat tokens
NCORES = 8
TPC = T // NCORES       # 512 tokens per core
C = 128                 # RNN chunk length
NCH = T // C            # 32 global chunks
NPAN = NCORES           # 8 token panels of 512
KD = D // 128           # 8 contraction chunks over D
NFF = FF // 128         # 22 ff chunks
G2W = 2 * DV + 1        # V | W | f projection group width
EPS = float(np.finfo(np.float32).eps)

_CACHE = {}


def _build(debug=False):
    nc = bacc.Bacc("TRN2", target_bir_lowering=False, debug=False,
                   num_devices=NCORES)
    f32, bf16 = dt.float32, dt.bfloat16

    # ---------------- I/O ----------------
    xT = nc.dram_tensor("xT", [D, TPC], f32, kind="ExternalInput")
    wqt = nc.dram_tensor("wqt", [D, DK], bf16, kind="ExternalInput")
    wkt = nc.dram_tensor("wkt", [D, DK], bf16, kind="ExternalInput")
    g2w = nc.dram_tensor("g2w", [D, G2W], bf16, kind="ExternalInput")
    wot = nc.dram_tensor("wot", [H * DV, D], bf16, kind="ExternalInput")
    w1t = nc.dram_tensor("w1t", [D, FF], bf16, kind="ExternalInput")
    w2t = nc.dram_tensor("w2t", [D, FF], bf16, kind="ExternalInput")
    w3t = nc.dram_tensor("w3t", [FF, D], bf16, kind="ExternalInput")
    yout = nc.dram_tensor("yout", [D, TPC], f32, kind="ExternalOutput")

    # constants
    tri_h = nc.inline_tensor(np.triu(np.full((C, C), 1.0, np.float32)),
                             name="tri_neg")
    mask_h = nc.inline_tensor(
        np.where(np.arange(C)[None, :] >= np.arange(C)[:, None],
                 0.0, -1e5).astype(np.float32), name="mask_c")
    eye_h = nc.inline_tensor(np.eye(C, dtype=np.float32), name="eye_f32")
    eyeb_h = nc.inline_tensor(np.eye(C, dtype=np.float32).astype(BF),
                              name="eye_bf16")
    onesrf_h = nc.inline_tensor(np.ones((1, C), np.float32), name="ones_r_f")
    onescb_h = nc.inline_tensor(np.ones((C, 1), BF), name="ones_c_b")
    eps_h = nc.inline_tensor(np.full((1, 1), EPS, np.float32), name="eps_c")

    with tile.TileContext(nc) as tc, \
            tc.tile_pool(name="pglob", bufs=1) as pglob, \
            tc.tile_pool(name="dramg", bufs=1, space="DRAM") as dramg:
        # ------- global persistent SBUF -------
        x2T_sb = pglob.tile([128, KD * TPC], f32, tag="x2T")
        hT_sb = pglob.tile([128, KD * TPC], bf16, tag="hT")
        gact = pglob.tile([128, NFF * TPC], bf16, tag="gact")
        tri_sb = pglob.tile([128, C], f32, tag="tri_sb")
        mask_sb = pglob.tile([128, C], f32, tag="mask_sb")
        eye_sb = pglob.tile([128, C], f32, tag="eye_sb")
        eyeb_sb = pglob.tile([128, C], bf16, tag="eyeb_sb")
        onr_f = pglob.tile([1, C], f32, tag="onr_f")
        onc_b = pglob.tile([128, 1], bf16, tag="onc_b")
        nc.sync.dma_start(tri_sb[:], tri_h[:])
        nc.sync.dma_start(mask_sb[:], mask_h[:])
        nc.sync.dma_start(eye_sb[:], eye_h[:])
        nc.sync.dma_start(eyeb_sb[:], eyeb_h[:])
        nc.sync.dma_start(onr_f[:], onesrf_h[:])
        nc.sync.dma_start(onc_b[:], onescb_h[:])
        eps_sb = pglob.tile([1, 1], f32, tag="eps_sb")
        nc.sync.dma_start(eps_sb[:], eps_h[:])
        og_in = dramg.tile([NCORES * 128, TPC], bf16)
        og_out = dramg.tile([NCORES * 128, TPC], bf16)

        with tc.tile_pool(name="p1", bufs=1) as p1:
            # ------- phase-1 persistent SBUF -------
            xT_sb = p1.tile([128, KD * TPC], f32, tag="xT")
            QT = p1.tile([128, T], bf16, tag="QT")
            KT = p1.tile([128, T], bf16, tag="KT")
            Qd = p1.tile([128, T], bf16, tag="Qd")
            Kb = p1.tile([128, T], bf16, tag="Kb")
            Kd = p1.tile([128, T], bf16, tag="Kd")
            Vb = p1.tile([128, T], bf16, tag="Vb")
            WGb = p1.tile([128, T], bf16, tag="WGb")
            Eall = p1.tile([128, T], bf16, tag="Eall")
            ogT = p1.tile([128, T], bf16, tag="ogT")
            sp_all = p1.tile([128, NCH], f32, tag="sp_all")
            Lcols = p1.tile([128, NCH], f32, tag="Lcols")
            kdec = p1.tile([128, NCH], f32, tag="kdec")
            bend = p1.tile([128, NCH], f32, tag="bend")

            # ============ scope A: norm1 + AllGather + projections ==========
            with (
                tc.tile_pool(name="wq_p", bufs=1) as wq_p,
                tc.tile_pool(name="gxn_p", bufs=2) as gxn_p,
                tc.tile_pool(name="dramA", bufs=1, space="DRAM") as dramA,
                tc.tile_pool(name="ps_A", bufs=1, space="PSUM") as ps_A,
                tc.tile_pool(name="sb_A", bufs=2) as sb_A,
            ):
                xnT_bounce = dramA.tile([D, TPC], bf16)
                gxn_dram = dramA.tile([NCORES * D, TPC], bf16,
                                      addr_space="Shared")
                sq_ps = ps_A.tile([1, TPC], f32, tag="sumsq")
                for kd in range(KD):
                    xt = xT_sb[:, kd * TPC:(kd + 1) * TPC]
                    nc.sync.dma_start(xt, xT[kd * 128:(kd + 1) * 128, :])
                    sqc = sb_A.tile([128, TPC], bf16, tag="sqc")
                    nc.scalar.activation(sqc[:], xt, AF.Square)
                    nc.tensor.matmul(sq_ps[:], onc_b[:], sqc[:],
                                     start=(kd == 0), stop=(kd == KD - 1))
                sqrt_sb = sb_A.tile([1, TPC], f32, tag="sqrt")
                nc.scalar.activation(sqrt_sb[:], sq_ps[:], AF.Sqrt,
                                     bias=eps_sb[:], scale=1.0 / D)
                rs_sb = sb_A.tile([1, TPC], f32, tag="rs")
                nc.vector.reciprocal(rs_sb[:], sqrt_sb[:])
                rsb_ps = ps_A.tile([128, TPC], f32, tag="rsb")
                nc.tensor.matmul(rsb_ps[:], onr_f[:], rs_sb[:],
                                 start=True, stop=True)
                for kd in range(KD):
                    xnc = sb_A.tile([128, TPC], bf16, tag="xnc")
                    nc.vector.tensor_mul(xnc[:],
                                         xT_sb[:, kd * TPC:(kd + 1) * TPC],
                                         rsb_ps[:])
                    nc.sync.dma_start(xnT_bounce[kd * 128:(kd + 1) * 128, :],
                                      xnc[:])

                nc.gpsimd.collective_compute(
                    "AllGather", ALU.bypass,
                    replica_groups=[list(range(NCORES))],
                    ins=[xnT_bounce[:]], outs=[gxn_dram[:]],
                )

                wq_sb = wq_p.tile([128, KD * DK], bf16, tag="wq")
                wk_sb = wq_p.tile([128, KD * DK], bf16, tag="wk")
                g2_sb = wq_p.tile([128, KD * G2W], bf16, tag="g2")
                for kd in range(KD):
                    nc.sync.dma_start(wq_sb[:, kd * DK:(kd + 1) * DK],
                                      wqt[kd * 128:(kd + 1) * 128, :])
                    nc.sync.dma_start(wk_sb[:, kd * DK:(kd + 1) * DK],
                                      wkt[kd * 128:(kd + 1) * 128, :])
                    nc.sync.dma_start(g2_sb[:, kd * G2W:(kd + 1) * G2W],
                                      g2w[kd * 128:(kd + 1) * 128, :])

                for r in range(NPAN):
                    gxt = []
                    for kd in range(KD):
                        t_ = gxn_p.tile([128, TPC], bf16, tag=f"gx{kd}")
                        nc.sync.dma_start(
                            t_[:],
                            gxn_dram[r * D + kd * 128:
                                     r * D + (kd + 1) * 128, :])
                        gxt.append(t_)
                    qps = ps_A.tile([128, TPC], f32, tag="qps")
                    kps = ps_A.tile([128, TPC], f32, tag="kps")
                    for kd in range(KD):
                        nc.tensor.matmul(qps[:],
                                         wq_sb[:, kd * DK:(kd + 1) * DK],
                                         gxt[kd][:], start=(kd == 0),
                                         stop=(kd == KD - 1))
                    for kd in range(KD):
                        nc.tensor.matmul(kps[:],
                                         wk_sb[:, kd * DK:(kd + 1) * DK],
                                         gxt[kd][:], start=(kd == 0),
                                         stop=(kd == KD - 1))
                    nc.scalar.copy(QT[:, r * TPC:(r + 1) * TPC], qps[:])
                    nc.vector.tensor_copy(KT[:, r * TPC:(r + 1) * TPC], kps[:])
                    for tch in range(4):
                        g = r * 4 + tch
                        vps = ps_A.tile([128, G2W], f32, tag="vps", bufs=2)
                        for kd in range(KD):
                            nc.tensor.matmul(
                                vps[:],
                                gxt[kd][:, tch * 128:(tch + 1) * 128],
                                g2_sb[:, kd * G2W:(kd + 1) * G2W],
                                start=(kd == 0), stop=(kd == KD - 1))
                        nc.vector.tensor_copy(Vb[:, g * 128:(g + 1) * 128],
                                              vps[:, 0:DV])
                        nc.scalar.activation(WGb[:, g * 128:(g + 1) * 128],
                                             vps[:, DV:2 * DV], AF.Silu)
                        nc.scalar.activation(sp_all[:, g:g + 1],
                                             vps[:, 2 * DV:2 * DV + 1],
                                             AF.Copy)
                    for tch in range(4):
                        g = r * 4 + tch
                        ktr = ps_A.tile([128, 128], bf16, tag="ktr", bufs=2)
                        nc.tensor.transpose(ktr[:],
                                            KT[:, g * 128:(g + 1) * 128],
                                            eyeb_sb[:])
                        nc.vector.tensor_copy(Kb[:, g * 128:(g + 1) * 128],
                                              ktr[:])

            # ============ scope B: decay prep ============
            with (
                tc.tile_pool(name="ps_B", bufs=2, space="PSUM") as ps_B,
                tc.tile_pool(name="sb_B", bufs=3) as sb_B,
            ):
                f_all = sb_B.tile([128, NCH], f32, tag="f_all", bufs=1)
                nc.scalar.activation(f_all[:], sp_all[:], AF.Sigmoid)
                lf_all = sb_B.tile([128, NCH], f32, tag="lf_all", bufs=1)
                nc.scalar.activation(lf_all[:], f_all[:], AF.Ln)
                lc_ps = ps_B.tile([128, NCH], f32, tag="lc", bufs=1)
                nc.tensor.matmul(lc_ps[:], tri_sb[:], lf_all[:],
                                 start=True, stop=True)
                nc.vector.tensor_copy(Lcols[:], lc_ps[:])

                for g in range(NCH):
                    lcol = Lcols[:, g:g + 1]
                    lr_ps = ps_B.tile([1, C], f32, tag="lr", bufs=1)
                    nc.tensor.matmul(lr_ps[:], lcol, eye_sb[:],
                                     start=True, stop=True)
                    lrow = sb_B.tile([1, C], f32, tag="lrow")
                    nc.vector.tensor_copy(lrow[:], lr_ps[:])
                    draw = ps_B.tile([128, C], f32, tag="draw")
                    nc.tensor.matmul(draw[:], onr_f[:], lrow[:],
                                     start=True, stop=True)
                    dm = sb_B.tile([128, C], f32, tag="dm")
                    nc.vector.scalar_tensor_tensor(
                        dm[:], draw[:], lcol, mask_sb[:],
                        op0=ALU.subtract, op1=ALU.add)
                    nc.scalar.activation(Eall[:, g * 128:(g + 1) * 128],
                                         dm[:], AF.Exp)
                    exprow = sb_B.tile([1, C], f32, tag="exprow")
                    nc.scalar.activation(exprow[:], lrow[:], AF.Exp)
                    explb = ps_B.tile([128, C], f32, tag="explb")
                    nc.tensor.matmul(explb[:], onr_f[:], exprow[:],
                                     start=True, stop=True)
                    nc.vector.tensor_mul(Qd[:, g * 128:(g + 1) * 128],
                                         QT[:, g * 128:(g + 1) * 128],
                                         explb[:])
                    lendb = ps_B.tile([128, 1], f32, tag="lendb", bufs=1)
                    nc.tensor.matmul(lendb[:], onr_f[:],
                                     lrow[0:1, C - 1:C], start=True, stop=True)
                    nc.scalar.activation(bend[:, g:g + 1], lendb[:], AF.Exp)
                    kds = sb_B.tile([128, 1], f32, tag="kds")
                    nc.vector.tensor_sub(kds[:], lendb[:], lcol)
                    nc.scalar.activation(kdec[:, g:g + 1], kds[:], AF.Exp)
                    nc.scalar.activation(Kd[:, g * 128:(g + 1) * 128],
                                         Kb[:, g * 128:(g + 1) * 128],
                                         AF.Copy, scale=kdec[:, g:g + 1])

            # ============ scope C: the RNN scan + AllToAll ============
            with (
                tc.tile_pool(name="ps_C", bufs=2, space="PSUM") as ps_C,
                tc.tile_pool(name="sb_C", bufs=3) as sb_C,
                tc.tile_pool(name="sb_S", bufs=2) as sb_S,
            ):
                Sf = [None, None]
                Sbf = [None, None]
                for b_ in range(B):
                    sf = sb_S.tile([128, 128], f32, tag=f"Sf{b_}")
                    nc.vector.memset(sf[:], 0.0)
                    Sf[b_] = sf
                for ci in range(16):
                    for b_ in range(B):
                        g = b_ * 16 + ci
                        cs = slice(g * 128, (g + 1) * 128)
                        pt = ps_C.tile([128, 128], f32, tag="pt")
                        nc.tensor.matmul(pt[:], KT[:, cs], QT[:, cs],
                                         start=True, stop=True)
                        pte = sb_C.tile([128, 128], bf16, tag="pte")
                        nc.vector.tensor_mul(pte[:], pt[:], Eall[:, cs])
                        ops = ps_C.tile([128, 128], f32, tag="ops")
                        nc.tensor.matmul(ops[:], pte[:], Vb[:, cs],
                                         start=True, stop=(ci == 0))
                        if ci > 0:
                            nc.tensor.matmul(ops[:], Qd[:, cs], Sbf[b_][:],
                                             start=False, stop=True)
                        og = sb_C.tile([128, 128], f32, tag="og")
                        nc.vector.tensor_mul(og[:], ops[:], WGb[:, cs])
                        ogt_ps = ps_C.tile([128, 128], f32, tag="ogt")
                        nc.tensor.transpose(ogt_ps[:], og[:], eye_sb[:])
                        nc.scalar.copy(ogT[:, cs], ogt_ps[:])
                        sps = ps_C.tile([128, 128], f32, tag="sps")
                        nc.tensor.matmul(sps[:], Kd[:, cs], Vb[:, cs],
                                         start=True, stop=True)
                        sf_new = sb_S.tile([128, 128], f32, tag=f"Sf{b_}")
                        nc.vector.scalar_tensor_tensor(
                            sf_new[:], Sf[b_][:], bend[:, g:g + 1], sps[:],
                            op0=ALU.mult, op1=ALU.add)
                        Sf[b_] = sf_new
                        if ci < 15:
                            sb_ = sb_S.tile([128, 128], bf16, tag=f"Sb{b_}")
                            nc.scalar.copy(sb_[:], sf_new[:])
                            Sbf[b_] = sb_

                for j in range(NCORES):
                    nc.sync.dma_start(og_in[j * 128:(j + 1) * 128, :],
                                      ogT[:, j * TPC:(j + 1) * TPC])
                nc.gpsimd.collective_compute(
                    "AllToAll", ALU.bypass,
                    replica_groups=[list(range(NCORES))],
                    ins=[og_in[:]], outs=[og_out[:]],
                )

                if debug:
                    for nm, t_ in (("dbg_QT", QT), ("dbg_Eall", Eall),
                                   ("dbg_ogT", ogT), ("dbg_Kd", Kd)):
                        d_ = nc.dram_tensor(nm, [128, T], bf16,
                                            kind="ExternalOutput")
                        nc.sync.dma_start(d_[:], t_[:])
                    d_ = nc.dram_tensor("dbg_sp", [128, NCH], f32,
                                        kind="ExternalOutput")
                    nc.sync.dma_start(d_[:], sp_all[:])
                    d_ = nc.dram_tensor("dbg_Lc", [128, NCH], f32,
                                        kind="ExternalOutput")
                    nc.sync.dma_start(d_[:], Lcols[:])

            # ============ scope D: Wo + residual + norm2 ============
            with (
                tc.tile_pool(name="wo_p", bufs=1) as wo_p,
                tc.tile_pool(name="ps_D", bufs=2, space="PSUM") as ps_D,
                tc.tile_pool(name="sb_D", bufs=2) as sb_D,
            ):
                wo_sb = []
                gog_sb = []
                for kh in range(H):
                    wt_ = wo_p.tile([128, D], bf16, tag=f"wo{kh}")
                    nc.sync.dma_start(wt_[:], wot[kh * 128:(kh + 1) * 128, :])
                    wo_sb.append(wt_)
                    gt_ = wo_p.tile([128, TPC], bf16, tag=f"gog{kh}")
                    nc.sync.dma_start(gt_[:],
                                      og_out[kh * 128:(kh + 1) * 128, :])
                    gog_sb.append(gt_)
                sq2_ps = ps_D.tile([1, TPC], f32, tag="sumsq2", bufs=1)
                for dch in range(KD):
                    mx = ps_D.tile([128, TPC], f32, tag="mx")
                    for kh in range(H):
                        nc.tensor.matmul(
                            mx[:], wo_sb[kh][:, dch * 128:(dch + 1) * 128],
                            gog_sb[kh][:], start=(kh == 0),
                            stop=(kh == H - 1))
                    x2c = x2T_sb[:, dch * TPC:(dch + 1) * TPC]
                    nc.vector.tensor_add(x2c, mx[:],
                                         xT_sb[:, dch * TPC:(dch + 1) * TPC])
                    sq2 = sb_D.tile([128, TPC], bf16, tag="sq2")
                    nc.scalar.activation(sq2[:], x2c, AF.Square)
                    nc.tensor.matmul(sq2_ps[:], onc_b[:], sq2[:],
                                     start=(dch == 0), stop=(dch == KD - 1))
                sqrt2 = sb_D.tile([1, TPC], f32, tag="sqrt2")
                nc.scalar.activation(sqrt2[:], sq2_ps[:], AF.Sqrt,
                                     bias=eps_sb[:], scale=1.0 / D)
                rs2 = sb_D.tile([1, TPC], f32, tag="rs2")
                nc.vector.reciprocal(rs2[:], sqrt2[:])
                rs2b = ps_D.tile([128, TPC], f32, tag="rs2b", bufs=1)
                nc.tensor.matmul(rs2b[:], onr_f[:], rs2[:],
                                 start=True, stop=True)
                for dch in range(KD):
                    nc.vector.tensor_mul(hT_sb[:, dch * TPC:(dch + 1) * TPC],
                                         x2T_sb[:, dch * TPC:(dch + 1) * TPC],
                                         rs2b[:])
                if debug:
                    d2 = nc.dram_tensor("dbg_x2T", [D, TPC], f32,
                                        kind="ExternalOutput")
                    for dch in range(KD):
                        nc.sync.dma_start(d2[dch * 128:(dch + 1) * 128, :],
                                          x2T_sb[:, dch * TPC:(dch + 1) * TPC])
            # p1 closes here: phase-1 tensors freed

        # ============ scope E: SwiGLU MLP ============
        with (
            tc.tile_pool(name="w1_p", bufs=1) as w1_p,
            tc.tile_pool(name="ps_E1", bufs=2, space="PSUM") as ps_E1,
        ):
            w1_sb = []
            for kd in range(KD):
                t1 = w1_p.tile([128, FF], bf16, tag=f"w1{kd}")
                nc.sync.dma_start(t1[:], w1t[kd * 128:(kd + 1) * 128, :])
                w1_sb.append(t1)
            for fch in range(NFF):
                u1 = ps_E1.tile([128, TPC], f32, tag="u1")
                for kd in range(KD):
                    nc.tensor.matmul(u1[:],
                                     w1_sb[kd][:, fch * 128:(fch + 1) * 128],
                                     hT_sb[:, kd * TPC:(kd + 1) * TPC],
                                     start=(kd == 0), stop=(kd == KD - 1))
                nc.scalar.activation(gact[:, fch * TPC:(fch + 1) * TPC],
                                     u1[:], AF.Silu)
        with (
            tc.tile_pool(name="w2_p", bufs=1) as w2_p,
            tc.tile_pool(name="ps_E2", bufs=2, space="PSUM") as ps_E2,
        ):
            w2_sb = []
            for kd in range(KD):
                t2 = w2_p.tile([128, FF], bf16, tag=f"w2{kd}")
                nc.sync.dma_start(t2[:], w2t[kd * 128:(kd + 1) * 128, :])
                w2_sb.append(t2)
            for fch in range(NFF):
                u2 = ps_E2.tile([128, TPC], f32, tag="u2")
                for kd in range(KD):
                    nc.tensor.matmul(u2[:],
                                     w2_sb[kd][:, fch * 128:(fch + 1) * 128],
                                     hT_sb[:, kd * TPC:(kd + 1) * TPC],
                                     start=(kd == 0), stop=(kd == KD - 1))
                gc_ = gact[:, fch * TPC:(fch + 1) * TPC]
                nc.vector.tensor_mul(gc_, gc_, u2[:])
        with (
            tc.tile_pool(name="w3_p", bufs=4) as w3_p,
            tc.tile_pool(name="ps_E3", bufs=1, space="PSUM") as ps_E3,
            tc.tile_pool(name="sb_E3", bufs=2) as sb_E3,
        ):
            yp = [ps_E3.tile([128, TPC], f32, tag=f"yp{dch}", name=f"yp{dch}")
                  for dch in range(KD)]
            for fch in range(NFF):
                t3 = w3_p.tile([128, D], bf16, tag="w3")
                nc.sync.dma_start(t3[:], w3t[fch * 128:(fch + 1) * 128, :])
                for dch in range(KD):
                    nc.tensor.matmul(yp[dch][:],
                                     t3[:, dch * 128:(dch + 1) * 128],
                                     gact[:, fch * TPC:(fch + 1) * TPC],
                                     start=(fch == 0), stop=(fch == NFF - 1))
            for dch in range(KD):
                oc = sb_E3.tile([128, TPC], f32, tag="oc")
                nc.vector.tensor_add(oc[:], yp[dch][:],
                                     x2T_sb[:, dch * TPC:(dch + 1) * TPC])
                nc.sync.dma_start(yout[dch * 128:(dch + 1) * 128, :], oc[:])

    nc.compile()
    return nc


def _host_prep(inputs):
    x = np.asarray(inputs["x"], np.float32)
    n1 = np.asarray(inputs["norm1_w"], np.float32)
    n2 = np.asarray(inputs["norm2_w"], np.float32)
    xT = np.ascontiguousarray(x.reshape(T, D).T)          # [D, T]
    sc = np.float32(DK ** -0.5)
    WqT = np.ascontiguousarray(inputs["Wq"]).T * (n1[:, None] * sc)
    WkT = np.ascontiguousarray(inputs["Wk"]).T * n1[:, None]
    WvT = np.ascontiguousarray(inputs["Wv"]).T * n1[:, None]
    WwT = np.ascontiguousarray(inputs["Ww"]).T * n1[:, None]
    WfT = np.ascontiguousarray(inputs["Wf"]).T * n1[:, None]    # [D, H]
    WoT = np.ascontiguousarray(np.asarray(inputs["Wo"]).T.astype(BF))
    w1T = np.ascontiguousarray((np.asarray(inputs["w1"]).T
                                * n2[:, None]).astype(BF))
    w2T = np.ascontiguousarray((np.asarray(inputs["w2"]).T
                                * n2[:, None]).astype(BF))
    w3T = np.ascontiguousarray(np.asarray(inputs["w3"]).T.astype(BF))

    in_maps = []
    for c in range(NCORES):
        hs = slice(c * DK, (c + 1) * DK)
        g2 = np.concatenate(
            [WvT[:, c * DV:(c + 1) * DV], WwT[:, c * DV:(c + 1) * DV],
             WfT[:, c:c + 1]], axis=1).astype(BF)
        in_maps.append({
            "xT": np.ascontiguousarray(xT[:, c * TPC:(c + 1) * TPC]),
            "wqt": np.ascontiguousarray(WqT[:, hs].astype(BF)),
            "wkt": np.ascontiguousarray(WkT[:, hs].astype(BF)),
            "g2w": np.ascontiguousarray(g2),
            "wot": WoT,
            "w1t": w1T,
            "w2t": w2T,
            "w3t": w3T,
        })
    return in_maps


def _run(inputs, debug=False):
    key = ("nc", debug)
    if key not in _CACHE:
        _CACHE[key] = _build(debug=debug)
    nc = _CACHE[key]
    in_maps = _host_prep(inputs)
    res = bass_utils.run_bass_kernel_spmd(
        nc, in_maps, core_ids=list(range(NCORES)), trace=False)
    return res


def kernel(**inputs):
    res = _run(inputs, debug=False)
    outT = np.concatenate([res.results[c]["yout"] for c in range(NCORES)],
                          axis=1)                          # [D, T]
    return np.ascontiguousarray(outT.T).reshape(B, S, D).astype(np.float32)
